# revision 43
# baseline (speedup 1.0000x reference)
"""Multi-head self-attention Trainium2 kernel (8 NeuronCores, SPMD).

Problem: x[2, 4096, 512], 8 heads, dq=64.
  q = x@Wq.T+bq ; k = x@Wk.T+bk ; v = x@Wv.T+bv
  att = softmax(q k^T / sqrt(64)) ; out = (att v) @ Wo.T + bo

Sharding: batch (2) x query-quarter (4) -> 8 cores. Core c handles batch
c//4, query rows [(c%4)*1024, (c%4+1)*1024). Every core computes full K/V
for its batch (duplicated 4x, cheap), attention for all 8 heads over its
1024 query rows, and the output projection for its rows. The host only
concatenates per-core outputs -- no cross-core reduction needed.

On-device layout notes (per core):
  - Inputs are pre-cast to bf16 on the host (the kernel computes in bf16
    with fp32 PSUM accumulation; measured rel err vs fp32 ref ~1.6e-3).
  - All matmuls contract over the partition dim; the host hands each core
    its inputs already transposed (d on partitions) and tiled to the exact
    SBUF layouts, so the kernel preamble is a handful of contiguous DMAs
    (layout prep is part of the host-side sharding).
  - Scores are computed transposed, S.T[tk, tq], per head: lhsT = K_h.T
    (dq=64 contraction; even/odd heads live on partitions 0-63/64-127 and
    are issued as adjacent pairs -- enforced with no-sync dep edges -- so
    the second LDWEIGHTS overlaps the first matmul's stream).
  - exp runs on ScalarE straight out of PSUM into SBUF (bf16), with the
    1/sqrt(dq) scale folded into the activation's scale. ScalarE splits
    PSUM reads at bank granularity (512 fp32), making it, with PE, the
    co-bottleneck (~33.5M exps per core).
  - att@V uses V with an appended ones-column: the extra output row is the
    softmax denominator, so no separate reduction pass is needed.
  - Normalization: the denominator row is copied PSUM->SBUF (the custom-DVE
    fast reciprocal reads garbage from PSUM), reciprocal'd on VectorE,
    GpSimd partition-broadcast, VectorE multiply.
"""

import numpy as np


import concourse.bacc as bacc
import concourse.mybir as mybir
import concourse.tile as tile
from concourse.bass_utils import run_bass_kernel_spmd



F32 = mybir.dt.float32
BF16 = mybir.dt.bfloat16

B = 2
T = 4096
D = 512
H = 8
DQ = 64
TQ = 1024  # query rows per core
NCORES = 8
NHP = 4  # head pairs
G = 2  # tk tiles (of 128) per exp group: 2 PSUM banks per score tile


def _build_program():
    nc = bacc.Bacc(None)

    # inputs arrive pre-transposed (d on partitions) and bf16 -- layout prep
    # happens host-side in kernel() below
    xt = nc.declare_dram_parameter("xt", [128, 16, TQ], BF16, isOutput=False)
    xqt = nc.declare_dram_parameter("xqt", [128, 4, TQ], BF16, isOutput=False)
    ws = {
        name: nc.declare_dram_parameter(name, [128, 4, D], BF16, isOutput=False)
        for name in ("WqT", "WkT", "WvT", "WoT")
    }
    bs = {
        name: nc.declare_dram_parameter(name, [D], F32, isOutput=False)
        for name in ("bq", "bk", "bv", "bo")
    }
    y = nc.declare_dram_parameter("y", [TQ, D], F32, isOutput=True)

    with tile.TileContext(nc) as tc:
        _emit(nc, tc, xt, xqt, ws, bs, y)
    if not nc.is_finalized():
        nc.finalize()
    return nc


def _emit(nc, tc, xt, xqt, ws, bs, y):
    from contextlib import ExitStack

    ctx = ExitStack()
    with ctx:
        persist = ctx.enter_context(tc.tile_pool(name="persist", bufs=1))

        # persistent SBUF tensors
        # x[batch].T in 4 chunks of 1024 t-rows (loaded independently so
        # projections can start before the whole x.T has landed)
        XTc = [
            persist.tile([128, 4, 1024], BF16, name=f"XT{c}") for c in range(4)
        ]
        XQT = persist.tile([128, 4, TQ], BF16)  # xq.T
        WqT = persist.tile([128, 4, D], BF16)
        WkT = persist.tile([128, 4, D], BF16)
        WvT = persist.tile([128, 4, D], BF16)
        WoT = persist.tile([128, 4, D], BF16)
        KT = persist.tile([128, NHP, T], BF16)  # K.T per head-pair
        QT = persist.tile([128, NHP, TQ], BF16)
        VH = persist.tile([128, T // 128, H, DQ + 1], BF16)  # [V | ones]
        AOT = persist.tile([128, NHP, TQ], BF16)  # normalized (att@V).T
        bq_s = persist.tile([128, 4], F32)
        bk_s = persist.tile([128, 4], F32)
        bv_b = persist.tile([128, D], F32)
        bo_b = persist.tile([128, D], F32)
        # per-partition bias layouts for Q.T/K.T (bias indexed by j = partition)
        nc.sync.dma_start(out=bq_s, in_=bs["bq"].rearrange("(a p) -> p a", p=128))
        nc.sync.dma_start(out=bk_s, in_=bs["bk"].rearrange("(a p) -> p a", p=128))
        # broadcast bias layouts for V / y (bias indexed by j = free dim)
        nc.sync.dma_start(
            out=bv_b,
            in_=bs["bv"].rearrange("(a d) -> a d", a=1).to_broadcast((128, D)),
        )
        nc.sync.dma_start(
            out=bo_b,
            in_=bs["bo"].rearrange("(a d) -> a d", a=1).to_broadcast((128, D)),
        )
        nc.vector.memset(VH[:, :, :, DQ : DQ + 1], 1.0)

        wts = {"WqT": WqT, "WkT": WkT, "WvT": WvT, "WoT": WoT}

        # ---- Stage A/B: straight DMA loads (inputs pre-transposed on host) ----
        # order by first use: WkT + XTc[0] gate the first K-projection
        nc.scalar.dma_start(out=WkT, in_=ws["WkT"][:, :, :])
        nc.scalar.dma_start(out=WqT, in_=ws["WqT"][:, :, :])
        nc.scalar.dma_start(out=XQT, in_=xqt[:, :, :])
        nc.scalar.dma_start(out=WvT, in_=ws["WvT"][:, :, :])
        nc.scalar.dma_start(out=WoT, in_=ws["WoT"][:, :, :])
        for c in range(4):
            nc.sync.dma_start(out=XTc[c], in_=xt[:, c * 4 : (c + 1) * 4, :])

        # ---- Stage C: projections ----
        with tc.tile_pool(name="pp", bufs=8, space="PSUM") as pp:
            # K.T[j, t] and Q.T[j, t]: lhsT = W.T[d, j-tile], rhs = X.T[d, t-chunk]
            for jt in range(4):
                for ch in range(T // 512):
                    kp = pp.tile([128, 512], F32, tag="proj")
                    for dc in range(4):
                        nc.tensor.matmul(
                            kp,
                            lhsT=WkT[:, dc, jt * 128 : (jt + 1) * 128],
                            rhs=XTc[ch // 2][:, dc, (ch % 2) * 512 : (ch % 2 + 1) * 512],
                            start=(dc == 0),
                            stop=(dc == 3),
                        )
                    nc.vector.tensor_scalar_add(
                        out=KT[:, jt, ch * 512 : (ch + 1) * 512],
                        in0=kp,
                        scalar1=bk_s[:, jt : jt + 1],
                    )
                for ch in range(TQ // 512):
                    qp = pp.tile([128, 512], F32, tag="proj")
                    for dc in range(4):
                        nc.tensor.matmul(
                            qp,
                            lhsT=WqT[:, dc, jt * 128 : (jt + 1) * 128],
                            rhs=XQT[:, dc, ch * 512 : (ch + 1) * 512],
                            start=(dc == 0),
                            stop=(dc == 3),
                        )
                    nc.vector.tensor_scalar_add(
                        out=QT[:, jt, ch * 512 : (ch + 1) * 512],
                        in0=qp,
                        scalar1=bq_s[:, jt : jt + 1],
                    )
            # V[t, j]: lhsT = X.T[d, t-tile], rhs = Wv.T[d, :]
            for tt in range(T // 128):
                vp = pp.tile([128, 512], F32, tag="proj")
                for dc in range(4):
                    nc.tensor.matmul(
                        vp,
                        lhsT=XTc[tt // 8][:, dc, (tt % 8) * 128 : (tt % 8 + 1) * 128],
                        rhs=WvT[:, dc, :],
                        start=(dc == 0),
                        stop=(dc == 3),
                    )
                nc.vector.tensor_add(
                    out=VH[:, tt, :, 0:DQ],
                    in0=vp.rearrange("p (h v) -> p h v", h=H),
                    in1=bv_b.rearrange("p (h v) -> p h v", h=H),
                )

        # ---- Stage D: attention ----
        ngroups = (T // 128 + G - 1) // G
        with (
            tc.tile_pool(name="sp", bufs=3, space="PSUM") as sp,
            tc.tile_pool(name="op", bufs=2, space="PSUM") as op,
            tc.tile_pool(name="se", bufs=6) as se,
            tc.tile_pool(name="epi", bufs=4) as epi,
        ):
            for qc in range(TQ // 512):
                qsl = slice(qc * 512, (qc + 1) * 512)
                for hp in range(NHP):
                    attO = [
                        op.tile([DQ + 1, 512], F32, tag="attO", name=f"attO{p}")
                        for p in range(2)
                    ]
                    tkt = 0
                    for g in range(ngroups):
                        gsz = min(G, T // 128 - tkt)
                        sc = [
                            sp.tile([128, G, 512], F32, tag="sc", name=f"sc{p}")
                            for p in range(2)
                        ]
                        # interleave even/odd head score matmuls: adjacent
                        # K=64 matmuls on disjoint PE row-groups overlap.
                        # Chain them with no-sync dep edges so the Tile
                        # scheduler preserves the alternation.
                        for i in range(gsz):
                            tksl = slice((tkt + i) * 128, (tkt + i + 1) * 128)
                            mm_pair = []
                            for par in range(2):
                                psl = slice(par * 64, (par + 1) * 64)
                                mm = nc.tensor.matmul(
                                    sc[par][:, i, :],
                                    lhsT=KT[psl, hp, tksl],
                                    rhs=QT[psl, hp, qsl],
                                    start=True,
                                    stop=True,
                                )
                                mm_pair.append(mm)
                            tile.add_dep_helper(
                                mm_pair[1].ins, mm_pair[0].ins, False, "pair order"
                            )
                        for par in range(2):
                            h = hp * 2 + par
                            ex = se.tile([128, G, 512], BF16, tag="ex")
                            nc.scalar.activation(
                                out=ex[:, :gsz, :],
                                in_=sc[par][:, :gsz, :],
                                func=mybir.ActivationFunctionType.Exp,
                                scale=float(DQ) ** -0.5,
                            )
                            for i in range(gsz):
                                nc.tensor.matmul(
                                    attO[par],
                                    lhsT=VH[:, tkt + i, h, :],
                                    rhs=ex[:, i, :],
                                    start=(tkt + i == 0),
                                    stop=(tkt + i == T // 128 - 1),
                                    skip_group_check=True,
                                )
                        tkt += gsz
                    for par in range(2):
                        den_s = epi.tile([1, 512], F32, tag="den_s")
                        nc.vector.tensor_copy(
                            out=den_s, in_=attO[par][DQ : DQ + 1, :]
                        )
                        rden = epi.tile([1, 512], F32, tag="rden")
                        nc.vector.reciprocal_approx_fast(out=rden, in_=den_s)
                        bc = epi.tile([64, 512], F32, tag="bc")
                        nc.gpsimd.partition_broadcast(out_ap=bc, in_ap=rden)
                        nc.vector.tensor_mul(
                            out=AOT[par * 64 : (par + 1) * 64, hp, qsl],
                            in0=attO[par][0:DQ, :],
                            in1=bc,
                        )

        # ---- Stage E: output projection ----
        with (
            tc.tile_pool(name="yp", bufs=4, space="PSUM") as yp,
            tc.tile_pool(name="yo", bufs=4) as yo,
        ):
            for tt in range(TQ // 128):
                ypt = yp.tile([128, 512], F32, tag="y")
                for hp in range(NHP):
                    nc.tensor.matmul(
                        ypt,
                        lhsT=AOT[:, hp, tt * 128 : (tt + 1) * 128],
                        rhs=WoT[:, hp, :],
                        start=(hp == 0),
                        stop=(hp == NHP - 1),
                    )
                ys = yo.tile([128, 512], F32, tag="ys")
                nc.vector.tensor_add(out=ys, in0=ypt, in1=bo_b)
                nc.sync.dma_start(out=y[tt * 128 : (tt + 1) * 128, :], in_=ys)


_CACHED = {}


def _get_program():
    if "nc" not in _CACHED:
        _CACHED["nc"] = _build_program()
    return _CACHED["nc"]


def kernel(x, Wq, bq, Wk, bk, Wv, bv, Wo, bo, _trace=False):
    import ml_dtypes

    bf16 = ml_dtypes.bfloat16

    def wt_layout(w):
        # W [j, d] f32 -> W.T [128, 4, D] bf16 with d = dc*128 + p
        wb = np.asarray(w, dtype=np.float32).astype(bf16)
        return np.ascontiguousarray(wb.T.reshape(4, 128, D).transpose(1, 0, 2))

    x = np.asarray(x, dtype=np.float32).astype(bf16)
    weights = {
        "WqT": wt_layout(Wq),
        "WkT": wt_layout(Wk),
        "WvT": wt_layout(Wv),
        "WoT": wt_layout(Wo),
        "bq": np.ascontiguousarray(np.asarray(bq, dtype=np.float32)),
        "bk": np.ascontiguousarray(np.asarray(bk, dtype=np.float32)),
        "bv": np.ascontiguousarray(np.asarray(bv, dtype=np.float32)),
        "bo": np.ascontiguousarray(np.asarray(bo, dtype=np.float32)),
    }
    # x[b].T [128, 16, TQ]: [p, c*4+dc, tl] = x[b].T[dc*128+p, c*1024+tl]
    xts = [
        np.ascontiguousarray(
            x[b].T.reshape(4, 128, 4, TQ).transpose(1, 2, 0, 3).reshape(128, 16, TQ)
        )
        for b in range(B)
    ]

    nc = _get_program()
    in_maps = []
    for c in range(NCORES):
        b = c // 4
        q0 = (c % 4) * TQ
        xqt = np.ascontiguousarray(
            x[b, q0 : q0 + TQ].T.reshape(4, 128, TQ).transpose(1, 0, 2)
        )
        in_maps.append({"xt": xts[b], "xqt": xqt, **weights})

    res = run_bass_kernel_spmd(
        nc, in_maps, core_ids=list(range(NCORES)), trace=_trace
    )

    out = np.empty((B, T, D), dtype=np.float32)
    for c in range(NCORES):
        b = c // 4
        q0 = (c % 4) * TQ
        out[b, q0 : q0 + TQ] = res.results[c]["y"]
    if _trace:
        return out, res
    return out


# revision 44
# speedup vs baseline: 1.0381x; 1.0381x over previous
"""Multi-head self-attention Trainium2 kernel (8 NeuronCores, SPMD).

Problem: x[2, 4096, 512], 8 heads, dq=64.
  q = x@Wq.T+bq ; k = x@Wk.T+bk ; v = x@Wv.T+bv
  att = softmax(q k^T / sqrt(64)) ; out = (att v) @ Wo.T + bo

Sharding: batch (2) x query-quarter (4) -> 8 cores. Core c handles batch
c//4, query rows [(c%4)*1024, (c%4+1)*1024). Every core computes full K/V
for its batch (duplicated 4x, cheap), attention for all 8 heads over its
1024 query rows, and the output projection for its rows. The host only
concatenates per-core outputs -- no cross-core reduction needed.

On-device layout notes (per core):
  - Inputs are pre-cast to bf16 on the host (the kernel computes in bf16
    with fp32 PSUM accumulation; measured rel err vs fp32 ref ~1.6e-3).
  - All matmuls contract over the partition dim; the host hands each core
    its inputs already transposed (d on partitions) and tiled to the exact
    SBUF layouts, so the kernel preamble is a handful of contiguous DMAs
    (layout prep is part of the host-side sharding).
  - Scores are computed transposed, S.T[tk, tq], per head: lhsT = K_h.T
    (dq=64 contraction; even/odd heads live on partitions 0-63/64-127 and
    are issued as adjacent pairs -- enforced with no-sync dep edges -- so
    the second LDWEIGHTS overlaps the first matmul's stream).
  - exp runs on ScalarE straight out of PSUM into SBUF (bf16), with the
    1/sqrt(dq) scale folded into the activation's scale. ScalarE splits
    PSUM reads at bank granularity (512 fp32), making it, with PE, the
    co-bottleneck (~33.5M exps per core).
  - att@V uses V with an appended ones-column: the extra output row is the
    softmax denominator, so no separate reduction pass is needed.
  - Normalization: the denominator row is copied PSUM->SBUF (the custom-DVE
    fast reciprocal reads garbage from PSUM), reciprocal'd on VectorE,
    GpSimd partition-broadcast, VectorE multiply.
"""

import numpy as np


import concourse.bacc as bacc
import concourse.mybir as mybir
import concourse.tile as tile
from concourse.bass_utils import run_bass_kernel_spmd



F32 = mybir.dt.float32
BF16 = mybir.dt.bfloat16

B = 2
T = 4096
D = 512
H = 8
DQ = 64
TQ = 1024  # query rows per core
NCORES = 8
NHP = 4  # head pairs
G = 2  # tk tiles (of 128) per exp group: 2 PSUM banks per score tile


def _build_program():
    nc = bacc.Bacc(None)

    # inputs arrive pre-transposed (d on partitions) and bf16 -- layout prep
    # happens host-side in kernel() below
    xt = nc.declare_dram_parameter("xt", [128, 16, TQ], BF16, isOutput=False)
    xqt = nc.declare_dram_parameter("xqt", [128, 4, TQ], BF16, isOutput=False)
    ws = {
        name: nc.declare_dram_parameter(name, [128, 4, D], BF16, isOutput=False)
        for name in ("WqT", "WkT", "WvT", "WoT")
    }
    bs = {
        name: nc.declare_dram_parameter(name, [D], F32, isOutput=False)
        for name in ("bq", "bk", "bv", "bo")
    }
    y = nc.declare_dram_parameter("y", [TQ, D], F32, isOutput=True)

    with tile.TileContext(nc) as tc:
        _emit(nc, tc, xt, xqt, ws, bs, y)
    if not nc.is_finalized():
        nc.finalize()
    return nc


def _emit(nc, tc, xt, xqt, ws, bs, y):
    from contextlib import ExitStack

    ctx = ExitStack()
    with ctx:
        persist = ctx.enter_context(tc.tile_pool(name="persist", bufs=1))

        # persistent SBUF tensors
        # x[batch].T in 4 chunks of 1024 t-rows (loaded independently so
        # projections can start before the whole x.T has landed)
        XTc = [
            persist.tile([128, 4, 1024], BF16, name=f"XT{c}") for c in range(4)
        ]
        XQT = persist.tile([128, 4, TQ], BF16)  # xq.T
        WqT = persist.tile([128, 4, D], BF16)
        WkT = persist.tile([128, 4, D], BF16)
        WvT = persist.tile([128, 4, D], BF16)
        WoT = persist.tile([128, 4, D], BF16)
        KT = persist.tile([128, NHP, T], BF16)  # K.T per head-pair
        QT = persist.tile([128, NHP, TQ], BF16)
        VH = persist.tile([128, T // 128, H, DQ + 1], BF16)  # [V | ones]
        AOT = persist.tile([128, NHP, TQ], BF16)  # normalized (att@V).T
        bq_s = persist.tile([128, 4], F32)
        bk_s = persist.tile([128, 4], F32)
        bv_b = persist.tile([128, D], F32)
        bo_b = persist.tile([128, D], F32)
        # per-partition bias layouts for Q.T/K.T (bias indexed by j = partition)
        nc.sync.dma_start(out=bq_s, in_=bs["bq"].rearrange("(a p) -> p a", p=128))
        nc.sync.dma_start(out=bk_s, in_=bs["bk"].rearrange("(a p) -> p a", p=128))
        # broadcast bias layouts for V / y (bias indexed by j = free dim)
        nc.sync.dma_start(
            out=bv_b,
            in_=bs["bv"].rearrange("(a d) -> a d", a=1).to_broadcast((128, D)),
        )
        nc.sync.dma_start(
            out=bo_b,
            in_=bs["bo"].rearrange("(a d) -> a d", a=1).to_broadcast((128, D)),
        )
        nc.vector.memset(VH[:, :, :, DQ : DQ + 1], 1.0)

        wts = {"WqT": WqT, "WkT": WkT, "WvT": WvT, "WoT": WoT}

        # ---- Stage A/B: straight DMA loads (inputs pre-transposed on host) ----
        # order by first use: WkT + XTc[0] gate the first K-projection
        nc.scalar.dma_start(out=WkT, in_=ws["WkT"][:, :, :])
        nc.scalar.dma_start(out=WqT, in_=ws["WqT"][:, :, :])
        nc.scalar.dma_start(out=XQT, in_=xqt[:, :, :])
        nc.scalar.dma_start(out=WvT, in_=ws["WvT"][:, :, :])
        nc.scalar.dma_start(out=WoT, in_=ws["WoT"][:, :, :])
        for c in range(4):
            nc.sync.dma_start(out=XTc[c], in_=xt[:, c * 4 : (c + 1) * 4, :])

        # ---- Stage C: projections, in 4 interleaved waves ----
        # wave w emits K.T/Q.T for head-pair w AND V for t-tiles 8w..8w+7,
        # so attention for head-pair 0 (scores, exp, att@V) can start while
        # later head-pairs are still projecting.
        with tc.tile_pool(name="pp", bufs=8, space="PSUM") as pp:
            for w in range(4):
                jt = w
                for ch in range(T // 512):
                    kp = pp.tile([128, 512], F32, tag="proj")
                    for dc in range(4):
                        nc.tensor.matmul(
                            kp,
                            lhsT=WkT[:, dc, jt * 128 : (jt + 1) * 128],
                            rhs=XTc[ch // 2][:, dc, (ch % 2) * 512 : (ch % 2 + 1) * 512],
                            start=(dc == 0),
                            stop=(dc == 3),
                        )
                    nc.vector.tensor_scalar_add(
                        out=KT[:, jt, ch * 512 : (ch + 1) * 512],
                        in0=kp,
                        scalar1=bk_s[:, jt : jt + 1],
                    )
                for ch in range(TQ // 512):
                    qp = pp.tile([128, 512], F32, tag="proj")
                    for dc in range(4):
                        nc.tensor.matmul(
                            qp,
                            lhsT=WqT[:, dc, jt * 128 : (jt + 1) * 128],
                            rhs=XQT[:, dc, ch * 512 : (ch + 1) * 512],
                            start=(dc == 0),
                            stop=(dc == 3),
                        )
                    nc.vector.tensor_scalar_add(
                        out=QT[:, jt, ch * 512 : (ch + 1) * 512],
                        in0=qp,
                        scalar1=bq_s[:, jt : jt + 1],
                    )
                # V[t, j]: lhsT = X.T[d, t-tile], rhs = Wv.T[d, :]
                for tt in range(w * 8, (w + 1) * 8):
                    vp = pp.tile([128, 512], F32, tag="proj")
                    for dc in range(4):
                        nc.tensor.matmul(
                            vp,
                            lhsT=XTc[tt // 8][:, dc, (tt % 8) * 128 : (tt % 8 + 1) * 128],
                            rhs=WvT[:, dc, :],
                            start=(dc == 0),
                            stop=(dc == 3),
                        )
                    nc.vector.tensor_add(
                        out=VH[:, tt, :, 0:DQ],
                        in0=vp.rearrange("p (h v) -> p h v", h=H),
                        in1=bv_b.rearrange("p (h v) -> p h v", h=H),
                    )

        # ---- Stage D: attention ----
        ngroups = (T // 128 + G - 1) // G
        with (
            tc.tile_pool(name="sp", bufs=3, space="PSUM") as sp,
            tc.tile_pool(name="op", bufs=2, space="PSUM") as op,
            tc.tile_pool(name="se", bufs=6) as se,
            tc.tile_pool(name="epi", bufs=4) as epi,
        ):
            for qc in range(TQ // 512):
                qsl = slice(qc * 512, (qc + 1) * 512)
                for hp in range(NHP):
                    attO = [
                        op.tile([DQ + 1, 512], F32, tag="attO", name=f"attO{p}")
                        for p in range(2)
                    ]
                    tkt = 0
                    for g in range(ngroups):
                        gsz = min(G, T // 128 - tkt)
                        sc = [
                            sp.tile([128, G, 512], F32, tag="sc", name=f"sc{p}")
                            for p in range(2)
                        ]
                        # interleave even/odd head score matmuls: adjacent
                        # K=64 matmuls on disjoint PE row-groups overlap.
                        # Chain them with no-sync dep edges so the Tile
                        # scheduler preserves the alternation.
                        for i in range(gsz):
                            tksl = slice((tkt + i) * 128, (tkt + i + 1) * 128)
                            mm_pair = []
                            for par in range(2):
                                psl = slice(par * 64, (par + 1) * 64)
                                mm = nc.tensor.matmul(
                                    sc[par][:, i, :],
                                    lhsT=KT[psl, hp, tksl],
                                    rhs=QT[psl, hp, qsl],
                                    start=True,
                                    stop=True,
                                )
                                mm_pair.append(mm)
                            tile.add_dep_helper(
                                mm_pair[1].ins, mm_pair[0].ins, False, "pair order"
                            )
                        for par in range(2):
                            h = hp * 2 + par
                            ex = se.tile([128, G, 512], BF16, tag="ex")
                            nc.scalar.activation(
                                out=ex[:, :gsz, :],
                                in_=sc[par][:, :gsz, :],
                                func=mybir.ActivationFunctionType.Exp,
                                scale=float(DQ) ** -0.5,
                            )
                            for i in range(gsz):
                                nc.tensor.matmul(
                                    attO[par],
                                    lhsT=VH[:, tkt + i, h, :],
                                    rhs=ex[:, i, :],
                                    start=(tkt + i == 0),
                                    stop=(tkt + i == T // 128 - 1),
                                    skip_group_check=True,
                                )
                        tkt += gsz
                    for par in range(2):
                        den_s = epi.tile([1, 512], F32, tag="den_s")
                        nc.vector.tensor_copy(
                            out=den_s, in_=attO[par][DQ : DQ + 1, :]
                        )
                        rden = epi.tile([1, 512], F32, tag="rden")
                        nc.vector.reciprocal_approx_fast(out=rden, in_=den_s)
                        bc = epi.tile([64, 512], F32, tag="bc")
                        nc.gpsimd.partition_broadcast(out_ap=bc, in_ap=rden)
                        nc.vector.tensor_mul(
                            out=AOT[par * 64 : (par + 1) * 64, hp, qsl],
                            in0=attO[par][0:DQ, :],
                            in1=bc,
                        )

        # ---- Stage E: output projection ----
        with (
            tc.tile_pool(name="yp", bufs=4, space="PSUM") as yp,
            tc.tile_pool(name="yo", bufs=4) as yo,
        ):
            for tt in range(TQ // 128):
                ypt = yp.tile([128, 512], F32, tag="y")
                for hp in range(NHP):
                    nc.tensor.matmul(
                        ypt,
                        lhsT=AOT[:, hp, tt * 128 : (tt + 1) * 128],
                        rhs=WoT[:, hp, :],
                        start=(hp == 0),
                        stop=(hp == NHP - 1),
                    )
                ys = yo.tile([128, 512], F32, tag="ys")
                nc.vector.tensor_add(out=ys, in0=ypt, in1=bo_b)
                nc.sync.dma_start(out=y[tt * 128 : (tt + 1) * 128, :], in_=ys)


_CACHED = {}


def _get_program():
    if "nc" not in _CACHED:
        _CACHED["nc"] = _build_program()
    return _CACHED["nc"]


def kernel(x, Wq, bq, Wk, bk, Wv, bv, Wo, bo, _trace=False):
    import ml_dtypes

    bf16 = ml_dtypes.bfloat16

    def wt_layout(w):
        # W [j, d] f32 -> W.T [128, 4, D] bf16 with d = dc*128 + p
        wb = np.asarray(w, dtype=np.float32).astype(bf16)
        return np.ascontiguousarray(wb.T.reshape(4, 128, D).transpose(1, 0, 2))

    x = np.asarray(x, dtype=np.float32).astype(bf16)
    weights = {
        "WqT": wt_layout(Wq),
        "WkT": wt_layout(Wk),
        "WvT": wt_layout(Wv),
        "WoT": wt_layout(Wo),
        "bq": np.ascontiguousarray(np.asarray(bq, dtype=np.float32)),
        "bk": np.ascontiguousarray(np.asarray(bk, dtype=np.float32)),
        "bv": np.ascontiguousarray(np.asarray(bv, dtype=np.float32)),
        "bo": np.ascontiguousarray(np.asarray(bo, dtype=np.float32)),
    }
    # x[b].T [128, 16, TQ]: [p, c*4+dc, tl] = x[b].T[dc*128+p, c*1024+tl]
    xts = [
        np.ascontiguousarray(
            x[b].T.reshape(4, 128, 4, TQ).transpose(1, 2, 0, 3).reshape(128, 16, TQ)
        )
        for b in range(B)
    ]

    nc = _get_program()
    in_maps = []
    for c in range(NCORES):
        b = c // 4
        q0 = (c % 4) * TQ
        xqt = np.ascontiguousarray(
            x[b, q0 : q0 + TQ].T.reshape(4, 128, TQ).transpose(1, 0, 2)
        )
        in_maps.append({"xt": xts[b], "xqt": xqt, **weights})

    res = run_bass_kernel_spmd(
        nc, in_maps, core_ids=list(range(NCORES)), trace=_trace
    )

    out = np.empty((B, T, D), dtype=np.float32)
    for c in range(NCORES):
        b = c // 4
        q0 = (c % 4) * TQ
        out[b, q0 : q0 + TQ] = res.results[c]["y"]
    if _trace:
        return out, res
    return out
